# revision 15
# baseline (speedup 1.0000x reference)
"""Trainium2 Bass kernel: 4096x4096 fp32 image, 7x7 valid cross-correlation + bias.

Strategy
--------
Column-shard the image across 8 NeuronCores: core m computes output columns
[512*m, 512*m+512) (tail cropped on host; every core sees 512 columns + 6 halo
columns of input, zero-padded to 520).

2D-tiled Toeplitz: the 128 SBUF partitions carry a 16x8 image patch,
p = 8*a + b  <->  x[16*s + a, 8*q + b]  (slab s along the free axis, q-tile of
8 columns).  One matmul contracts a full patch against a stationary
S[(a,b), (i,j)] = w[a-i, b-j] producing 128 output pixels (i,j) per streamed
column -- 2x the useful density of the 1D banded-Toeplitz form.  Taps that
cross the patch boundary (i+di >= 16 row-wrap, j+dj >= 8 col-wrap) are handled
by three more matmuls whose moving operand is the same SBUF buffer shifted by
one slab (free offset +65) and/or one q-tile (free offset +1), accumulated in
the same PSUM bank via start/stop.  Total: 4 matmuls of 512 free-columns per
128x512 output chunk = 2048 PE cycles per 65536 outputs, ~2x faster than the
7-matmul row-band form.

Outputs are evicted PSUM->SBUF with a fused bias add (scalar/vector engines
alternate) and stored as fp16 (halves store traffic; |err| ~ 5e-4 rel, gate is
2e-2).  Inputs are fp16 (PE streams 16-bit at full rate, PSUM accumulates
fp32).  Loads ride the Sync HWDGE ring, stores the GpSimd ring.
"""

import os
import sys

import numpy as np

for _p in ("/root/.axon_site/_ro/trn_rl_repo", "/opt/trn_rl_repo"):
    if os.path.isdir(_p) and _p not in sys.path:
        sys.path.append(_p)

H = W = 4096
KH = KW = 7
OH = OW = H - KH + 1            # 4090
NCORES = 8
CW = 512                        # output columns per core
A, B = 16, 8                    # patch rows x cols (A*B = 128 partitions)
QT = 65                         # q-tiles per slab (65*8 = 520 >= 512+6)
NSLAB = 257                     # 16-row slabs (4112 rows incl. zero tail)
ROWS_PAD = NSLAB * A            # 4112
COLS_PAD = QT * B               # 520
SPC = 8                         # slabs per chunk (128 output rows)
NCHUNK = 32                     # chunks per core
NGRP = 4                        # chunk groups (8 PSUM banks each)
# chunks per output DMA: batches of 4 while compute hides them, per-chunk at
# the tail so the last store is tiny
STOREB = [4, 4, 4, 4, 4, 4, 4, 1, 1, 1, 1]

_prog = None


def _program():
    global _prog
    if _prog is not None:
        return _prog

    from contextlib import ExitStack

    import concourse.bass as bass
    import concourse.tile as tile
    from concourse import bacc, mybir

    nc = bacc.Bacc("TRN2", target_bir_lowering=False, debug=False)
    xs = nc.dram_tensor(
        "xs", [128, NSLAB, QT], mybir.dt.float16, kind="ExternalInput"
    )
    ws = nc.dram_tensor("ws", [128, 4, 128], mybir.dt.float16, kind="ExternalInput")
    br = nc.dram_tensor("br", [128, 1], mybir.dt.float32, kind="ExternalInput")
    yd = nc.dram_tensor(
        "yd", [128, NCHUNK, CW], mybir.dt.float16, kind="ExternalOutput"
    )
    xs_ap, ws_ap, br_ap, yd_ap = xs.ap(), ws.ap(), br.ap(), yd.ap()

    with tile.TileContext(nc) as tc, ExitStack() as ctx:
        consts = ctx.enter_context(tc.tile_pool(name="consts", bufs=1))
        xpool = ctx.enter_context(tc.tile_pool(name="xpool", bufs=1))
        pss = ctx.enter_context(tc.tile_pool(name="pss", bufs=8, space="PSUM"))
        ypool = ctx.enter_context(tc.tile_pool(name="ypool", bufs=1))

        # Everything the PE needs goes on ONE ring (sync) in strict need
        # order: stationaries first, then image granules (small first so
        # chunk 0 starts early).  Transfers on other rings get starved for
        # multiple us behind a busy sync ring, so only the bias (not needed
        # until the first eviction) rides scalar.
        w_t = consts.tile([128, 4, 128], mybir.dt.float16)
        nc.sync.dma_start(w_t[:, :, :], ws_ap[:, :, :])
        b_t = consts.tile([128, 1], mybir.dt.float32)
        nc.scalar.dma_start(b_t[:, :], br_ap)

        xall = xpool.tile([128, NSLAB, QT], mybir.dt.float16)
        sched = [(0, 8), (8, 16), (16, 24)]
        sched += [(s0, min(s0 + 16, NSLAB)) for s0 in range(24, NSLAB, 16)]
        for s0, s1 in sched:
            nc.sync.dma_start(xall[:, s0:s1, :], xs_ap[:, s0:s1, :])

        # burn the HAM cold window with junk matmuls while the first
        # granule's DMA is in flight (the junk weights need only the memset)
        junk = consts.tile([128, 128 + CW], mybir.dt.float16)
        nc.gpsimd.memset(junk[:, :], 0)
        wps = pss.tile([128, SPC, 64], mybir.dt.float32, tag="ps", name="warm")
        for _ in range(4):
            nc.tensor.matmul(
                wps[:, :, :],
                junk[:, 0:128],
                junk[:, 128 : 128 + CW],
                start=True,
                stop=True,
            )

        yo = ypool.tile([128, NCHUNK, CW], mybir.dt.float16)
        stores = []
        c_acc = 0
        for nb in STOREB:
            stores.append((c_acc, c_acc + nb))
            c_acc += nb

        # chunk-major: the 4 passes (row-wrap x col-wrap) of a chunk
        # accumulate back-to-back into its PSUM bank (canonical K-tiled
        # accumulate); LDWEIGHTS for the next pass hides in the background
        # weight buffer.  Evictions alternate scalar/vector.
        for c in range(NCHUNK):
            pt = pss.tile([128, SPC, 64], mybir.dt.float32, tag="ps", name=f"ps{c}")
            for si, (dt, dq) in enumerate([(0, 0), (0, 1), (1, 0), (1, 1)]):
                s0 = c * SPC + dt
                nc.tensor.matmul(
                    pt[:, :, :],
                    w_t[:, si, :],
                    xall[:, s0 : s0 + SPC, dq : dq + 64],
                    start=(si == 0),
                    stop=(si == 3),
                )
            if c % 2 == 0:
                nc.scalar.activation(
                    yo[:, c, :],
                    pt[:, :, :],
                    mybir.ActivationFunctionType.Identity,
                    bias=b_t[:, :],
                    scale=1.0,
                )
            else:
                nc.vector.tensor_scalar_add(yo[:, c, :], pt[:, :, :], b_t[:, :])
            while stores and stores[0][1] == c + 1:
                c0, c1 = stores.pop(0)
                st_eng = nc.sync if not stores else nc.gpsimd
                st_eng.dma_start(yd_ap[:, c0:c1, :], yo[:, c0:c1, :])


    nc.compile()
    _prog = nc
    return nc


def _shards(x, weight, bias):
    x = np.asarray(x, dtype=np.float32)
    weight = np.asarray(weight, dtype=np.float32)
    bias = np.asarray(bias, dtype=np.float32)

    xh = x.astype(np.float16)
    wh = weight.astype(np.float16)

    # stationaries: S[si=(2dt+dq)][8a+b, 8i+j] = w[a+16dt-i, b+8dq-j]
    S = np.zeros((128, 4, 128), dtype=np.float16)
    aa, bb, ii, jj = np.meshgrid(
        np.arange(A), np.arange(B), np.arange(A), np.arange(B), indexing="ij"
    )
    for si, (dt, dq) in enumerate([(0, 0), (0, 1), (1, 0), (1, 1)]):
        di = aa + 16 * dt - ii
        dj = bb + 8 * dq - jj
        m = (di >= 0) & (di < KH) & (dj >= 0) & (dj < KW)
        S[(aa * B + bb)[m], si, (ii * B + jj)[m]] = wh[di[m], dj[m]]

    brep = np.full((128, 1), np.float32(bias[0]), dtype=np.float32)

    ins = []
    for m in range(NCORES):
        xpad = np.zeros((ROWS_PAD, COLS_PAD), dtype=np.float16)
        c0 = m * CW
        c1 = min(c0 + CW + KW - 1, W)
        xpad[:H, : c1 - c0] = xh[:, c0:c1]
        # xs[8a+b, s, q] = xpad[16s+a, 8q+b]
        xsm = np.ascontiguousarray(
            xpad.reshape(NSLAB, A, QT, B).transpose(1, 3, 0, 2).reshape(128, NSLAB, QT)
        )
        ins.append({"xs": xsm, "ws": S, "br": brep})
    return ins


def _gather(results):
    y = np.empty((OH, OW), dtype=np.float32)
    for m in range(NCORES):
        c0 = m * CW
        c1 = min(c0 + CW, OW)
        # yd[8i+j, c, 64s+q] = out[16(8c+s)+i, 8q+j]
        full = (
            results[m]["yd"]
            .reshape(A, B, NCHUNK, SPC, 64)
            .transpose(2, 3, 0, 4, 1)
            .reshape(ROWS_PAD - A, CW)
        )
        y[:, c0:c1] = full[:OH, : c1 - c0].astype(np.float32)
    return y


def kernel(x, weight, bias):
    from concourse.bass_utils import run_bass_kernel_spmd

    nc = _program()
    in_maps = _shards(x, weight, bias)
    res = run_bass_kernel_spmd(nc, in_maps, core_ids=list(range(NCORES)))
    return _gather(res.results)


# revision 21
# speedup vs baseline: 1.0876x; 1.0876x over previous
"""Trainium2 Bass kernel: 4096x4096 fp32 image, 7x7 valid cross-correlation + bias.

Strategy
--------
Column-shard the image across 8 NeuronCores: core m computes output columns
[512*m, 512*m+512) (tail cropped on host; every core sees 512 columns + 6 halo
columns of input, zero-padded to 520).

2D-tiled Toeplitz: the 128 SBUF partitions carry a 16x8 image patch,
p = 8*a + b  <->  x[16*s + a, 8*q + b]  (slab s along the free axis, q-tile of
8 columns).  One matmul contracts a full patch against a stationary
S[(a,b), (i,j)] = w[a-i, b-j] producing 128 output pixels (i,j) per streamed
column -- 2x the useful density of the 1D banded-Toeplitz form.  Taps that
cross the patch boundary (i+di >= 16 row-wrap, j+dj >= 8 col-wrap) are handled
by three more matmuls whose moving operand is the same SBUF buffer shifted by
one slab (free offset +65) and/or one q-tile (free offset +1), accumulated in
the same PSUM bank via start/stop.  Total: 4 matmuls of 512 free-columns per
128x512 output chunk = 2048 PE cycles per 65536 outputs, ~2x faster than the
7-matmul row-band form.

Outputs are evicted PSUM->SBUF with a fused bias add (scalar/vector engines
alternate) and stored as fp16 (halves store traffic; |err| ~ 5e-4 rel, gate is
2e-2).  Inputs are fp16 (PE streams 16-bit at full rate, PSUM accumulates
fp32).  Loads ride the Sync HWDGE ring, stores the GpSimd ring.
"""

import os
import sys

import numpy as np

for _p in ("/root/.axon_site/_ro/trn_rl_repo", "/opt/trn_rl_repo"):
    if os.path.isdir(_p) and _p not in sys.path:
        sys.path.append(_p)

H = W = 4096
KH = KW = 7
OH = OW = H - KH + 1            # 4090
NCORES = 8
CW = 512                        # output columns per core
A, B = 16, 8                    # patch rows x cols (A*B = 128 partitions)
QT = 65                         # q-tiles per slab (65*8 = 520 >= 512+6)
NSLAB = 257                     # 16-row slabs (4112 rows incl. zero tail)
ROWS_PAD = NSLAB * A            # 4112
COLS_PAD = QT * B               # 520
SPC = 8                         # slabs per chunk (128 output rows)
NCHUNK = 32                     # chunks per core
NGRP = 4                        # chunk groups (8 PSUM banks each)
# chunks per output DMA: batches of 4 while compute hides them, per-chunk at
# the tail so the last store is tiny
STOREB = [4, 4, 4, 4, 4, 4, 4, 1, 1, 1, 1]

_prog = None


def _program():
    global _prog
    if _prog is not None:
        return _prog

    from contextlib import ExitStack

    import concourse.bass as bass
    import concourse.tile as tile
    from concourse import bacc, mybir

    nc = bacc.Bacc("TRN2", target_bir_lowering=False, debug=False)
    xs = nc.dram_tensor(
        "xs", [128, NSLAB, QT], mybir.dt.float16, kind="ExternalInput"
    )
    ws = nc.dram_tensor("ws", [128, 4, 128], mybir.dt.float16, kind="ExternalInput")
    br = nc.dram_tensor("br", [128, 1], mybir.dt.float32, kind="ExternalInput")
    yd = nc.dram_tensor(
        "yd", [128, NCHUNK, CW], mybir.dt.float16, kind="ExternalOutput"
    )
    xs_ap, ws_ap, br_ap, yd_ap = xs.ap(), ws.ap(), br.ap(), yd.ap()

    with tile.TileContext(nc) as tc, ExitStack() as ctx:
        consts = ctx.enter_context(tc.tile_pool(name="consts", bufs=1))
        xpool = ctx.enter_context(tc.tile_pool(name="xpool", bufs=1))
        pss = ctx.enter_context(tc.tile_pool(name="pss", bufs=8, space="PSUM"))
        ypool = ctx.enter_context(tc.tile_pool(name="ypool", bufs=1))

        # Everything the PE needs goes on ONE ring (sync) in strict need
        # order: stationaries first, then image granules (small first so
        # chunk 0 starts early).  Transfers on other rings get starved for
        # multiple us behind a busy sync ring, so only the bias (not needed
        # until the first eviction) rides scalar.
        w_t = consts.tile([128, 4, 128], mybir.dt.float16)
        nc.sync.dma_start(w_t[:, :, :], ws_ap[:, :, :])
        b_t = consts.tile([128, 1], mybir.dt.float32)
        nc.scalar.dma_start(b_t[:, :], br_ap)

        xall = xpool.tile([128, NSLAB, QT], mybir.dt.float16)
        sched = [(0, 8), (8, 16), (16, 24)]
        sched += [(s0, min(s0 + 16, NSLAB)) for s0 in range(24, NSLAB, 16)]
        for s0, s1 in sched:
            nc.sync.dma_start(xall[:, s0:s1, :], xs_ap[:, s0:s1, :])

        # Bridge the PE from program start to the first granule's arrival
        # with junk matmuls: the HAM clock gate needs ~3.4us of GAPLESS
        # activity to flip to 2.4 GHz, and an idle gap resets the streak.
        # ~7 junk MMs (~3us cold) end right as g0 lands, so the real stream
        # continues the streak and runs warm almost immediately.
        junk = consts.tile([128, 128 + CW], mybir.dt.float16)
        nc.gpsimd.memset(junk[:, :], 0)
        wps = pss.tile([128, SPC, 64], mybir.dt.float32, tag="ps", name="warm")
        for _ in range(7):
            nc.tensor.matmul(
                wps[:, :, :],
                junk[:, 0:128],
                junk[:, 128 : 128 + CW],
                start=True,
                stop=True,
            )

        yo = ypool.tile([128, NCHUNK, CW], mybir.dt.float16)
        stores = []
        c_acc = 0
        for nb in STOREB:
            stores.append((c_acc, c_acc + nb))
            c_acc += nb

        # chunk-major: the 4 passes (row-wrap x col-wrap) of a chunk
        # accumulate back-to-back into its PSUM bank (canonical K-tiled
        # accumulate); LDWEIGHTS for the next pass hides in the background
        # weight buffer.  Evictions alternate scalar/vector.
        for c in range(NCHUNK):
            pt = pss.tile([128, SPC, 64], mybir.dt.float32, tag="ps", name=f"ps{c}")
            for si, (dt, dq) in enumerate([(0, 0), (0, 1), (1, 0), (1, 1)]):
                s0 = c * SPC + dt
                nc.tensor.matmul(
                    pt[:, :, :],
                    w_t[:, si, :],
                    xall[:, s0 : s0 + SPC, dq : dq + 64],
                    start=(si == 0),
                    stop=(si == 3),
                )
            if c % 2 == 1:
                nc.scalar.activation(
                    yo[:, c, :],
                    pt[:, :, :],
                    mybir.ActivationFunctionType.Identity,
                    bias=b_t[:, :],
                    scale=1.0,
                )
            else:
                nc.vector.tensor_scalar_add(yo[:, c, :], pt[:, :, :], b_t[:, :])
            while stores and stores[0][1] == c + 1:
                c0, c1 = stores.pop(0)
                st_eng = nc.sync if not stores else nc.gpsimd
                st_eng.dma_start(yd_ap[:, c0:c1, :], yo[:, c0:c1, :])


    nc.compile()
    _prog = nc
    return nc


def _shards(x, weight, bias):
    x = np.asarray(x, dtype=np.float32)
    weight = np.asarray(weight, dtype=np.float32)
    bias = np.asarray(bias, dtype=np.float32)

    xh = x.astype(np.float16)
    wh = weight.astype(np.float16)

    # stationaries: S[si=(2dt+dq)][8a+b, 8i+j] = w[a+16dt-i, b+8dq-j]
    S = np.zeros((128, 4, 128), dtype=np.float16)
    aa, bb, ii, jj = np.meshgrid(
        np.arange(A), np.arange(B), np.arange(A), np.arange(B), indexing="ij"
    )
    for si, (dt, dq) in enumerate([(0, 0), (0, 1), (1, 0), (1, 1)]):
        di = aa + 16 * dt - ii
        dj = bb + 8 * dq - jj
        m = (di >= 0) & (di < KH) & (dj >= 0) & (dj < KW)
        S[(aa * B + bb)[m], si, (ii * B + jj)[m]] = wh[di[m], dj[m]]

    brep = np.full((128, 1), np.float32(bias[0]), dtype=np.float32)

    ins = []
    for m in range(NCORES):
        xpad = np.zeros((ROWS_PAD, COLS_PAD), dtype=np.float16)
        c0 = m * CW
        c1 = min(c0 + CW + KW - 1, W)
        xpad[:H, : c1 - c0] = xh[:, c0:c1]
        # xs[8a+b, s, q] = xpad[16s+a, 8q+b]
        xsm = np.ascontiguousarray(
            xpad.reshape(NSLAB, A, QT, B).transpose(1, 3, 0, 2).reshape(128, NSLAB, QT)
        )
        ins.append({"xs": xsm, "ws": S, "br": brep})
    return ins


def _gather(results):
    y = np.empty((OH, OW), dtype=np.float32)
    for m in range(NCORES):
        c0 = m * CW
        c1 = min(c0 + CW, OW)
        # yd[8i+j, c, 64s+q] = out[16(8c+s)+i, 8q+j]
        full = (
            results[m]["yd"]
            .reshape(A, B, NCHUNK, SPC, 64)
            .transpose(2, 3, 0, 4, 1)
            .reshape(ROWS_PAD - A, CW)
        )
        y[:, c0:c1] = full[:OH, : c1 - c0].astype(np.float32)
    return y


def kernel(x, weight, bias):
    from concourse.bass_utils import run_bass_kernel_spmd

    nc = _program()
    in_maps = _shards(x, weight, bias)
    res = run_bass_kernel_spmd(nc, in_maps, core_ids=list(range(NCORES)))
    return _gather(res.results)
